# revision 1
# baseline (speedup 1.0000x reference)
"""Trainium2 kernel for SparseLinear + bias + SELU (nn_AEEncoder).

Reference computation:
    y[b, o] = selu( sum_{e: out_idx[e]==o} weight[e] * x[b, in_idx[e]] + bias[o] )
with B=512, IN_F=20000, OUT_F=1000, NNZ=500000.

Strategy
--------
The edge list arrives as concrete numpy arrays at call time, so the sparse
weights are densified on the host into W[IN_F, OUT_F] (duplicate edges
accumulate). The device kernel is then a dense matmul y = x @ W + bias
followed by SELU, executed in bf16 (f32 PSUM accumulation).

Sharding: a 2-way batch x 4-way output-column grid over the 8 NeuronCores.
Each core computes a full [256, 250] block of the output independently —
no cross-core collectives (collectives pay a large all-core sync barrier
under this runner). The bias is folded into the matmul as one extra
contraction row (x^T gets a row of ones, W gets the bias row), so the
on-chip epilogue is only the SELU.

Per-core: x^T shard [20480, 256] bf16 + W shard [20480, 250] bf16 are
DMA-streamed to SBUF in k-tile groups; 320 accumulating matmuls
(2 M-tiles x 160 K-tiles, N=250) run concurrently with the DMA stream;
SELU is computed straight out of PSUM and the [256, 250] f32 block is
DMA'd out. The host assembles the 2x4 grid into the full [512, 1000].
"""

import numpy as np
import ml_dtypes

import concourse.bass as bass
import concourse.mybir as mybir
import concourse.tile as tile
from concourse import bacc
from concourse.bass_utils import run_bass_kernel_spmd

B, IN_F, OUT_F = 512, 20000, 1000
NCORES = 8
BS, OS = 2, 4          # batch split x out-column split (BS*OS == NCORES)
BSH = B // BS          # 256 batch rows per core
OSH = OUT_F // OS      # 250 output columns per core
KPAD = 20480           # padded contraction dim: 160 k-tiles of 128
KT = KPAD // 128       # 160 k-tiles (row IN_F==20000 carries the bias)
GROUP = 10             # k-tiles per DMA
MT = BSH // 128        # 2 M-tiles per core

SELU_SCALE = 1.0507009873554805
SELU_ALPHA = 1.6732632423543772

_compiled = None


def _build():
    nc = bacc.Bacc("TRN2", target_bir_lowering=False, debug=False,
                   num_devices=NCORES)
    xt_d = nc.dram_tensor("xt", [KT, 128, BSH], mybir.dt.bfloat16,
                          kind="ExternalInput")
    w_d = nc.dram_tensor("w", [KT, 128, OSH], mybir.dt.bfloat16,
                         kind="ExternalInput")
    out_d = nc.dram_tensor("out", [BSH, OSH], mybir.dt.float32,
                           kind="ExternalOutput")

    with tile.TileContext(nc) as tc:
        with (
            tc.tile_pool(name="sb", bufs=1) as sb,
            tc.tile_pool(name="ps", bufs=1, space="PSUM") as ps,
        ):
            xt_sb = sb.tile([128, KT * BSH], mybir.dt.bfloat16)
            w_sb = sb.tile([128, KT * OSH], mybir.dt.bfloat16)
            for g0 in range(0, KT, GROUP):
                g1 = min(KT, g0 + GROUP)
                nc.sync.dma_start(
                    xt_sb[:, g0 * BSH:g1 * BSH].rearrange(
                        "p (n m) -> p n m", m=BSH),
                    xt_d[g0:g1].rearrange("n p m -> p n m"),
                )
                nc.sync.dma_start(
                    w_sb[:, g0 * OSH:g1 * OSH].rearrange(
                        "p (n m) -> p n m", m=OSH),
                    w_d[g0:g1].rearrange("n p m -> p n m"),
                )

            accs = [
                ps.tile([128, OSH], mybir.dt.float32,
                        name=f"acc{m}", tag=f"acc{m}")
                for m in range(MT)
            ]
            for k in range(KT):
                for m in range(MT):
                    nc.tensor.matmul(
                        accs[m][:],
                        xt_sb[:, k * BSH + m * 128: k * BSH + (m + 1) * 128],
                        w_sb[:, k * OSH:(k + 1) * OSH],
                        start=(k == 0),
                        stop=(k == KT - 1),
                    )

            for m in range(MT):
                # selu(v) = scale*relu(v) + scale*alpha*(exp(min(v,0)) - 1)
                mn = sb.tile([128, OSH], mybir.dt.float32,
                             name=f"mn{m}", tag=f"mn{m}")
                nc.vector.tensor_scalar_min(mn[:], accs[m][:], 0.0)
                ex = sb.tile([128, OSH], mybir.dt.float32,
                             name=f"ex{m}", tag=f"ex{m}")
                nc.scalar.activation(ex[:], mn[:],
                                     mybir.ActivationFunctionType.Exp)
                rl = sb.tile([128, OSH], mybir.dt.float32,
                             name=f"rl{m}", tag=f"rl{m}")
                nc.scalar.activation(rl[:], accs[m][:],
                                     mybir.ActivationFunctionType.Relu,
                                     scale=SELU_SCALE)
                tt = sb.tile([128, OSH], mybir.dt.float32,
                             name=f"tt{m}", tag=f"tt{m}")
                nc.vector.tensor_scalar(tt[:], ex[:],
                                        SELU_SCALE * SELU_ALPHA,
                                        -SELU_SCALE * SELU_ALPHA,
                                        mybir.AluOpType.mult,
                                        mybir.AluOpType.add)
                oo = sb.tile([128, OSH], mybir.dt.float32,
                             name=f"oo{m}", tag=f"oo{m}")
                nc.vector.tensor_add(oo[:], rl[:], tt[:])
                nc.sync.dma_start(out_d[m * 128:(m + 1) * 128, :], oo[:])

    nc.compile()
    return nc


def _prepare_in_maps(x, weight, bias, out_idx, in_idx):
    x = np.asarray(x, dtype=np.float32)
    weight = np.asarray(weight, dtype=np.float32)
    bias = np.asarray(bias, dtype=np.float32)
    oi = np.asarray(out_idx).astype(np.int64)
    ii = np.asarray(in_idx).astype(np.int64)

    # densify the edge list; duplicate (i, o) pairs accumulate
    W = np.bincount(ii * OUT_F + oi, weights=weight.astype(np.float64),
                    minlength=IN_F * OUT_F).astype(np.float32)
    W = W.reshape(IN_F, OUT_F)

    Wp = np.zeros((KPAD, OUT_F), dtype=np.float32)
    Wp[:IN_F] = W
    Wp[IN_F] = bias           # bias row, matched by the ones row in x^T
    xtp = np.zeros((KPAD, B), dtype=np.float32)
    xtp[:IN_F] = x.T
    xtp[IN_F] = 1.0

    w_bf = Wp.astype(ml_dtypes.bfloat16)
    xt_bf = xtp.astype(ml_dtypes.bfloat16)

    in_maps = []
    for c in range(NCORES):
        b, o = divmod(c, OS)
        in_maps.append({
            "xt": np.ascontiguousarray(
                xt_bf[:, b * BSH:(b + 1) * BSH]).reshape(KT, 128, BSH),
            "w": np.ascontiguousarray(
                w_bf[:, o * OSH:(o + 1) * OSH]).reshape(KT, 128, OSH),
        })
    return in_maps


def _assemble(results):
    y = np.empty((B, OUT_F), dtype=np.float32)
    for c in range(NCORES):
        b, o = divmod(c, OS)
        y[b * BSH:(b + 1) * BSH, o * OSH:(o + 1) * OSH] = results[c]["out"]
    return y


def get_compiled():
    global _compiled
    if _compiled is None:
        _compiled = _build()
    return _compiled


def kernel(x, weight, bias, out_idx, in_idx):
    in_maps = _prepare_in_maps(x, weight, bias, out_idx, in_idx)
    nc = get_compiled()
    res = run_bass_kernel_spmd(nc, in_maps, core_ids=list(range(NCORES)))
    return _assemble(res.results)


# revision 5
# speedup vs baseline: 1.4722x; 1.4722x over previous
"""Trainium2 kernel for SparseLinear + bias + SELU (nn_AEEncoder).

Reference computation:
    y[b, o] = selu( sum_{e: out_idx[e]==o} weight[e] * x[b, in_idx[e]] + bias[o] )
with B=512, IN_F=20000, OUT_F=1000, NNZ=500000.

Strategy
--------
The edge list arrives as concrete numpy arrays at call time, so the sparse
weights are densified on the host into W[IN_F, OUT_F] (duplicate edges
accumulate). The device kernel is then a dense matmul y = x @ W + bias
followed by SELU, executed in bf16 (f32 PSUM accumulation).

Sharding: a 2-way batch x 4-way output-column grid over the 8 NeuronCores.
Each core computes a full [256, 250] block of the output independently —
no cross-core collectives (collectives pay a large all-core sync barrier
under this runner). The bias is folded into the matmul as one extra
contraction row (x^T gets a row of ones, W gets the bias row), so the
on-chip epilogue is only the SELU.

Per-core: x^T shard [20480, 256] bf16 + W shard [20480, 250] bf16 are
DMA-streamed to SBUF in k-tile groups; 320 accumulating matmuls
(2 M-tiles x 160 K-tiles, N=250) run concurrently with the DMA stream;
SELU is computed straight out of PSUM and the [256, 250] f32 block is
DMA'd out. The host assembles the 2x4 grid into the full [512, 1000].
"""

import numpy as np
import ml_dtypes

import concourse.bass as bass
import concourse.mybir as mybir
import concourse.tile as tile
from concourse import bacc
from concourse.bass_utils import run_bass_kernel_spmd

B, IN_F, OUT_F = 512, 20000, 1000
NCORES = 8
BS, OS = 2, 4          # batch split x out-column split (BS*OS == NCORES)
BSH = B // BS          # 256 batch rows per core
OSH = OUT_F // OS      # 250 output columns per core
KPAD = 20480           # padded contraction dim: 160 k-tiles of 128
KT = KPAD // 128       # 160 k-tiles (row IN_F==20000 carries the bias)
GROUP = 20             # k-tiles per DMA (~1.3 MB per transfer)
MT = BSH // 128        # 2 M-tiles per core

SELU_SCALE = 1.0507009873554805
SELU_ALPHA = 1.6732632423543772

_compiled = None


def _build():
    nc = bacc.Bacc("TRN2", target_bir_lowering=False, debug=False,
                   num_devices=NCORES)
    # partition-major layouts: row p holds that partition's whole k-stream,
    # so each DMA is 128 fully-contiguous runs (no small-packet penalty)
    xt_d = nc.dram_tensor("xt", [128, KT * BSH], mybir.dt.bfloat16,
                          kind="ExternalInput")
    w_d = nc.dram_tensor("w", [128, KT * OSH], mybir.dt.bfloat16,
                         kind="ExternalInput")
    out_d = nc.dram_tensor("out", [BSH, OSH], mybir.dt.float32,
                           kind="ExternalOutput")

    with tile.TileContext(nc) as tc:
        with (
            tc.tile_pool(name="sb", bufs=1) as sb,
            tc.tile_pool(name="ps", bufs=1, space="PSUM") as ps,
        ):
            xt_sb = sb.tile([128, KT * BSH], mybir.dt.bfloat16)
            w_sb = sb.tile([128, KT * OSH], mybir.dt.bfloat16)
            for g0 in range(0, KT, GROUP):
                g1 = min(KT, g0 + GROUP)
                nc.sync.dma_start(
                    xt_sb[:, g0 * BSH:g1 * BSH],
                    xt_d[:, g0 * BSH:g1 * BSH],
                )
                nc.scalar.dma_start(
                    w_sb[:, g0 * OSH:g1 * OSH],
                    w_d[:, g0 * OSH:g1 * OSH],
                )

            accs = [
                ps.tile([128, OSH], mybir.dt.float32,
                        name=f"acc{m}", tag=f"acc{m}")
                for m in range(MT)
            ]
            for k in range(KT):
                for m in range(MT):
                    nc.tensor.matmul(
                        accs[m][:],
                        xt_sb[:, k * BSH + m * 128: k * BSH + (m + 1) * 128],
                        w_sb[:, k * OSH:(k + 1) * OSH],
                        start=(k == 0),
                        stop=(k == KT - 1),
                    )

            for m in range(MT):
                # selu(v) = scale*relu(v) + scale*alpha*(exp(min(v,0)) - 1)
                mn = sb.tile([128, OSH], mybir.dt.float32,
                             name=f"mn{m}", tag=f"mn{m}")
                nc.vector.tensor_scalar_min(mn[:], accs[m][:], 0.0)
                ex = sb.tile([128, OSH], mybir.dt.float32,
                             name=f"ex{m}", tag=f"ex{m}")
                nc.scalar.activation(ex[:], mn[:],
                                     mybir.ActivationFunctionType.Exp)
                rl = sb.tile([128, OSH], mybir.dt.float32,
                             name=f"rl{m}", tag=f"rl{m}")
                nc.scalar.activation(rl[:], accs[m][:],
                                     mybir.ActivationFunctionType.Relu,
                                     scale=SELU_SCALE)
                tt = sb.tile([128, OSH], mybir.dt.float32,
                             name=f"tt{m}", tag=f"tt{m}")
                nc.vector.tensor_scalar(tt[:], ex[:],
                                        SELU_SCALE * SELU_ALPHA,
                                        -SELU_SCALE * SELU_ALPHA,
                                        mybir.AluOpType.mult,
                                        mybir.AluOpType.add)
                oo = sb.tile([128, OSH], mybir.dt.float32,
                             name=f"oo{m}", tag=f"oo{m}")
                nc.vector.tensor_add(oo[:], rl[:], tt[:])
                nc.sync.dma_start(out_d[m * 128:(m + 1) * 128, :], oo[:])

    nc.compile()
    return nc


def _prepare_in_maps(x, weight, bias, out_idx, in_idx):
    x = np.asarray(x, dtype=np.float32)
    weight = np.asarray(weight, dtype=np.float32)
    bias = np.asarray(bias, dtype=np.float32)
    oi = np.asarray(out_idx).astype(np.int64)
    ii = np.asarray(in_idx).astype(np.int64)

    # densify the edge list; duplicate (i, o) pairs accumulate
    W = np.bincount(ii * OUT_F + oi, weights=weight.astype(np.float64),
                    minlength=IN_F * OUT_F).astype(np.float32)
    W = W.reshape(IN_F, OUT_F)

    Wp = np.zeros((KPAD, OUT_F), dtype=np.float32)
    Wp[:IN_F] = W
    Wp[IN_F] = bias           # bias row, matched by the ones row in x^T
    xtp = np.zeros((KPAD, B), dtype=np.float32)
    xtp[:IN_F] = x.T
    xtp[IN_F] = 1.0

    w_bf = Wp.astype(ml_dtypes.bfloat16)
    xt_bf = xtp.astype(ml_dtypes.bfloat16)

    in_maps = []
    for c in range(NCORES):
        b, o = divmod(c, OS)
        xt_shard = np.ascontiguousarray(xt_bf[:, b * BSH:(b + 1) * BSH])
        w_shard = np.ascontiguousarray(w_bf[:, o * OSH:(o + 1) * OSH])
        in_maps.append({
            # -> partition-major [128, KT*cols]
            "xt": np.ascontiguousarray(
                xt_shard.reshape(KT, 128, BSH).transpose(1, 0, 2)
            ).reshape(128, KT * BSH),
            "w": np.ascontiguousarray(
                w_shard.reshape(KT, 128, OSH).transpose(1, 0, 2)
            ).reshape(128, KT * OSH),
        })
    return in_maps


def _assemble(results):
    y = np.empty((B, OUT_F), dtype=np.float32)
    for c in range(NCORES):
        b, o = divmod(c, OS)
        y[b * BSH:(b + 1) * BSH, o * OSH:(o + 1) * OSH] = results[c]["out"]
    return y


def get_compiled():
    global _compiled
    if _compiled is None:
        _compiled = _build()
    return _compiled


def kernel(x, weight, bias, out_idx, in_idx):
    in_maps = _prepare_in_maps(x, weight, bias, out_idx, in_idx)
    nc = get_compiled()
    res = run_bass_kernel_spmd(nc, in_maps, core_ids=list(range(NCORES)))
    return _assemble(res.results)


# revision 7
# speedup vs baseline: 1.5291x; 1.0386x over previous
"""Trainium2 kernel for SparseLinear + bias + SELU (nn_AEEncoder).

Reference computation:
    y[b, o] = selu( sum_{e: out_idx[e]==o} weight[e] * x[b, in_idx[e]] + bias[o] )
with B=512, IN_F=20000, OUT_F=1000, NNZ=500000.

Strategy
--------
The edge list arrives as concrete numpy arrays at call time, so the sparse
weights are densified on the host into W[IN_F, OUT_F] (duplicate edges
accumulate). The device kernel is then a dense matmul y = x @ W + bias
followed by SELU, executed in bf16 (f32 PSUM accumulation).

Sharding: a 2-way batch x 4-way output-column grid over the 8 NeuronCores.
Each core computes a full [256, 250] block of the output independently —
no cross-core collectives (collectives pay a large all-core sync barrier
under this runner). The bias is folded into the matmul as one extra
contraction row (x^T gets a row of ones, W gets the bias row), so the
on-chip epilogue is only the SELU.

Per-core: x^T shard [20480, 256] bf16 + W shard [20480, 250] bf16 are
DMA-streamed to SBUF in k-tile groups; 320 accumulating matmuls
(2 M-tiles x 160 K-tiles, N=250) run concurrently with the DMA stream;
SELU is computed straight out of PSUM and the [256, 250] f32 block is
DMA'd out. The host assembles the 2x4 grid into the full [512, 1000].
"""

import numpy as np
import ml_dtypes

import concourse.bass as bass
import concourse.mybir as mybir
import concourse.tile as tile
from concourse import bacc
from concourse.bass_utils import run_bass_kernel_spmd

B, IN_F, OUT_F = 512, 20000, 1000
NCORES = 8
BS, OS = 2, 4          # batch split x out-column split (BS*OS == NCORES)
BSH = B // BS          # 256 batch rows per core
OSH = OUT_F // OS      # 250 output columns per core
KPAD = 20480           # padded contraction dim: 160 k-tiles of 128
KT = KPAD // 128       # 160 k-tiles (row IN_F==20000 carries the bias)
# k-tiles per DMA group: small first group -> PE starts sooner; small last
# groups -> short compute tail after the final transfer. Sums to KT.
GROUPS = [10, 20, 20, 20, 20, 20, 20, 20, 6, 4]
MT = BSH // 128        # 2 M-tiles per core

SELU_SCALE = 1.0507009873554805
SELU_ALPHA = 1.6732632423543772

_compiled = None


def _build():
    nc = bacc.Bacc("TRN2", target_bir_lowering=False, debug=False,
                   num_devices=NCORES)
    # partition-major layouts: row p holds that partition's whole k-stream,
    # so each DMA is 128 fully-contiguous runs (no small-packet penalty)
    xt_d = nc.dram_tensor("xt", [128, KT * BSH], mybir.dt.bfloat16,
                          kind="ExternalInput")
    w_d = nc.dram_tensor("w", [128, KT * OSH], mybir.dt.bfloat16,
                         kind="ExternalInput")
    out_d = nc.dram_tensor("out", [BSH, OSH], mybir.dt.float32,
                           kind="ExternalOutput")

    with tile.TileContext(nc) as tc:
        with (
            tc.tile_pool(name="sb", bufs=1) as sb,
            tc.tile_pool(name="ps", bufs=1, space="PSUM") as ps,
        ):
            xt_sb = sb.tile([128, KT * BSH], mybir.dt.bfloat16)
            w_sb = sb.tile([128, KT * OSH], mybir.dt.bfloat16)
            assert sum(GROUPS) == KT
            g0 = 0
            for gsz in GROUPS:
                g1 = g0 + gsz
                nc.sync.dma_start(
                    xt_sb[:, g0 * BSH:g1 * BSH],
                    xt_d[:, g0 * BSH:g1 * BSH],
                )
                nc.scalar.dma_start(
                    w_sb[:, g0 * OSH:g1 * OSH],
                    w_d[:, g0 * OSH:g1 * OSH],
                )
                g0 = g1

            accs = [
                ps.tile([128, OSH], mybir.dt.float32,
                        name=f"acc{m}", tag=f"acc{m}")
                for m in range(MT)
            ]
            for k in range(KT):
                for m in range(MT):
                    nc.tensor.matmul(
                        accs[m][:],
                        xt_sb[:, k * BSH + m * 128: k * BSH + (m + 1) * 128],
                        w_sb[:, k * OSH:(k + 1) * OSH],
                        start=(k == 0),
                        stop=(k == KT - 1),
                    )

            for m in range(MT):
                # selu(v) = scale*relu(v) + scale*alpha*(exp(min(v,0)) - 1)
                mn = sb.tile([128, OSH], mybir.dt.float32,
                             name=f"mn{m}", tag=f"mn{m}")
                nc.vector.tensor_scalar_min(mn[:], accs[m][:], 0.0)
                ex = sb.tile([128, OSH], mybir.dt.float32,
                             name=f"ex{m}", tag=f"ex{m}")
                nc.scalar.activation(ex[:], mn[:],
                                     mybir.ActivationFunctionType.Exp)
                rl = sb.tile([128, OSH], mybir.dt.float32,
                             name=f"rl{m}", tag=f"rl{m}")
                nc.scalar.activation(rl[:], accs[m][:],
                                     mybir.ActivationFunctionType.Relu,
                                     scale=SELU_SCALE)
                tt = sb.tile([128, OSH], mybir.dt.float32,
                             name=f"tt{m}", tag=f"tt{m}")
                nc.vector.tensor_scalar(tt[:], ex[:],
                                        SELU_SCALE * SELU_ALPHA,
                                        -SELU_SCALE * SELU_ALPHA,
                                        mybir.AluOpType.mult,
                                        mybir.AluOpType.add)
                oo = sb.tile([128, OSH], mybir.dt.float32,
                             name=f"oo{m}", tag=f"oo{m}")
                nc.vector.tensor_add(oo[:], rl[:], tt[:])
                nc.sync.dma_start(out_d[m * 128:(m + 1) * 128, :], oo[:])

    nc.compile()
    return nc


def _prepare_in_maps(x, weight, bias, out_idx, in_idx):
    x = np.asarray(x, dtype=np.float32)
    weight = np.asarray(weight, dtype=np.float32)
    bias = np.asarray(bias, dtype=np.float32)
    oi = np.asarray(out_idx).astype(np.int64)
    ii = np.asarray(in_idx).astype(np.int64)

    # densify the edge list; duplicate (i, o) pairs accumulate
    W = np.bincount(ii * OUT_F + oi, weights=weight.astype(np.float64),
                    minlength=IN_F * OUT_F).astype(np.float32)
    W = W.reshape(IN_F, OUT_F)

    Wp = np.zeros((KPAD, OUT_F), dtype=np.float32)
    Wp[:IN_F] = W
    Wp[IN_F] = bias           # bias row, matched by the ones row in x^T
    xtp = np.zeros((KPAD, B), dtype=np.float32)
    xtp[:IN_F] = x.T
    xtp[IN_F] = 1.0

    w_bf = Wp.astype(ml_dtypes.bfloat16)
    xt_bf = xtp.astype(ml_dtypes.bfloat16)

    in_maps = []
    for c in range(NCORES):
        b, o = divmod(c, OS)
        xt_shard = np.ascontiguousarray(xt_bf[:, b * BSH:(b + 1) * BSH])
        w_shard = np.ascontiguousarray(w_bf[:, o * OSH:(o + 1) * OSH])
        in_maps.append({
            # -> partition-major [128, KT*cols]
            "xt": np.ascontiguousarray(
                xt_shard.reshape(KT, 128, BSH).transpose(1, 0, 2)
            ).reshape(128, KT * BSH),
            "w": np.ascontiguousarray(
                w_shard.reshape(KT, 128, OSH).transpose(1, 0, 2)
            ).reshape(128, KT * OSH),
        })
    return in_maps


def _assemble(results):
    y = np.empty((B, OUT_F), dtype=np.float32)
    for c in range(NCORES):
        b, o = divmod(c, OS)
        y[b * BSH:(b + 1) * BSH, o * OSH:(o + 1) * OSH] = results[c]["out"]
    return y


def get_compiled():
    global _compiled
    if _compiled is None:
        _compiled = _build()
    return _compiled


def kernel(x, weight, bias, out_idx, in_idx):
    in_maps = _prepare_in_maps(x, weight, bias, out_idx, in_idx)
    nc = get_compiled()
    res = run_bass_kernel_spmd(nc, in_maps, core_ids=list(range(NCORES)))
    return _assemble(res.results)
